# revision 35
# baseline (speedup 1.0000x reference)
"""Causal self-attention on 8 trn2 NeuronCores (bf16 datapath).

Problem: B=2, T=2048, C=1024, 16 heads of 64. Sharding: core = 4*b + g
(b = batch, g = head-group of 4 heads). Each core computes QKV projection
for its 4 heads, causal attention, and a partial c_proj (its 256 rows of
w_proj). Host sums the 4 partials per batch (the "all-reduce") + b_proj.

All SBUF operands are bf16 (PSUM accumulation stays fp32), which halves
input DMA bytes and removes the fp32r small-N matmul penalty, so the
S / AV matmuls can be trimmed to the causal boundary at 128-column
granularity.

Schedule: weights stream on the Activation HWDGE queue (wqk first),
x^T on the sync HWDGE queue; the pair-0 q/k projections consume x chunks
as they land, with zero-matmul warmups pinning the PE p-state ramp
through the DMA-paced stretch. Attention runs head-serial (one (pair,
half, h2) phase at a time), software-pipelined: S(t+1) and interleaved
filler (V tiles, pair-1 q/k quarter-projections, c_proj tiles) are
emitted before the AV work that parks on exp(t). AV runs flipped
(stationary P^T 128-col slice, moving V' -> out [tq, 65]) so each AV
matmul costs 65 PE cycles instead of ~512, and the softmax denominator
lands as a per-partition column: normalization is reciprocal +
tensor_scalar on DVE, then a PE transpose (identity moving operand)
rebuilds y^T for c_proj.

Per-core layouts (partition dim first):
  xT      (1024, 2048)  x[b]^T bf16; SBUF as 8 chunks (128, 2048)
  q^T/k^T (256, 2048)   bf16, 2 chunks each; chunk p = heads 2p, 2p+1;
                        1/sqrt(hs) folded into Wq,bq
  V'      (2048, 4, 65) bf16, natural + ones column (fused softmax
                        denominator: AV matmul emits [O^T; l] with M=65)
  S^T     (tk, tq) PSUM fp32; exp on ScalarE (no max subtraction: logits
                        ~N(0,1), exp cannot overflow); P^T bf16 in SBUF
  diag    strict-lower triangle of the diagonal 128x128 block is zeroed
                        by a bf16 upper-tri mask multiply on the DVE
  y^T     (256, 2048)   bf16 normalized attention out
  out     (2048, 1024)  bf16 partial y@w_proj; host sums in fp32
"""

import numpy as np
import ml_dtypes

import concourse.tile as tile
from concourse import bacc, mybir
from concourse.bass_utils import run_bass_kernel_spmd

B, T, C = 2, 2048, 1024
HS = 64
NCORES = 8
NHL = 4            # heads per core
TCH = 512          # tq / projection T chunk
NT = T // 128      # 16 tk tiles
F32 = mybir.dt.float32
BF16 = mybir.dt.bfloat16


def build_program():
    nc = bacc.Bacc("TRN2", target_bir_lowering=False, debug=False)

    xT_d = nc.dram_tensor("xT", [C, T], BF16, kind="ExternalInput").ap()
    wqk_d = nc.dram_tensor("wqk", [C, 512], BF16, kind="ExternalInput").ap()
    wv_d = nc.dram_tensor("wv", [C, 256], BF16, kind="ExternalInput").ap()
    wp_d = nc.dram_tensor("wp", [256, 1024], BF16, kind="ExternalInput").ap()
    bqk_d = nc.dram_tensor("bqk", [128, 4], F32, kind="ExternalInput").ap()
    bvb_d = nc.dram_tensor("bvb", [128, 320], BF16, kind="ExternalInput").ap()
    msk_d = nc.dram_tensor("msk", [128, 128], BF16, kind="ExternalInput").ap()
    idn_d = nc.dram_tensor("idn", [128, 128], BF16, kind="ExternalInput").ap()
    out_d = nc.dram_tensor("out", [T, C], BF16, kind="ExternalOutput").ap()

    with tile.TileContext(nc) as tc:
        _kernel(tc, out_d, xT_d, wqk_d, wv_d, wp_d, bqk_d, bvb_d, msk_d, idn_d)
    nc.compile()
    return nc


def _kernel(tc, out_d, xT_d, wqk_d, wv_d, wp_d, bqk_d, bvb_d, msk_d, idn_d):
    nc = tc.nc
    AF = mybir.ActivationFunctionType

    with (
        tc.tile_pool(name="persist", bufs=1) as pers,
        tc.tile_pool(name="ps", bufs=2, space="PSUM") as ps,
        tc.tile_pool(name="po", bufs=4, space="PSUM") as po,
    ):
        # Weights on the Activation HWDGE queue (bqk + wqk lead so the
        # first q-projection matmul only waits for wqk[0] + xt[0]); x^T
        # streams on the sync HWDGE queue in parallel.
        xp = tc.alloc_tile_pool(name="xp", bufs=1)
        xT3 = xT_d.rearrange("(c p) t -> c p t", p=128)
        wqk3 = wqk_d.rearrange("(c p) m -> c p m", p=128)
        wv3 = wv_d.rearrange("(c p) m -> c p m", p=128)
        wp3 = wp_d.rearrange("(c p) m -> c p m", p=128)
        xt, wqk, wv = [], [], []
        for c in range(8):
            w_ = pers.tile([128, 512], BF16, tag=f"wqk{c}", name=f"wqk{c}")
            nc.scalar.dma_start(out=w_, in_=wqk3[c])
            wqk.append(w_)
            t_ = xp.tile([128, T], BF16, tag=f"xt{c}", name=f"xt{c}")
            nc.sync.dma_start(out=t_, in_=xT3[c])
            xt.append(t_)
        bqk = pers.tile([128, 4], F32, tag="bqk")
        nc.scalar.dma_start(out=bqk, in_=bqk_d)
        for c in range(8):
            t_ = pers.tile([128, 256], BF16, tag=f"wv{c}", name=f"wv{c}")
            nc.scalar.dma_start(out=t_, in_=wv3[c])
            wv.append(t_)
        bvb = pers.tile([128, 320], BF16, tag="bvb")
        nc.scalar.dma_start(out=bvb, in_=bvb_d)
        msk = pers.tile([128, 128], BF16, tag="msk")
        nc.scalar.dma_start(out=msk, in_=msk_d)
        idn = pers.tile([128, 128], BF16, tag="idn")
        nc.scalar.dma_start(out=idn, in_=idn_d)
        wp = []
        for c in range(2):
            t_ = pers.tile([128, 1024], BF16, tag=f"wp{c}", name=f"wp{c}")
            nc.scalar.dma_start(out=t_, in_=wp3[c])
            wp.append(t_)

        # zero operands for PE-warmup matmuls: accumulate +0 into live
        # PSUM tiles purely to keep the tensor engine's p-state ramp hot
        # across DMA-paced stretches (an idle PE drops back to half clock
        # for the next 3us).
        zs = pers.tile([64, 128], BF16, tag="zs")
        nc.vector.memset(zs[:], 0)
        zs2 = pers.tile([64, 512], BF16, tag="zs2")
        nc.vector.memset(zs2[:], 0)

        def warm(pst, n=1, cols=512, first_start=False):
            for i in range(n):
                nc.tensor.matmul(
                    pst[0:128, 0:cols], zs[:], zs2[:, 0:cols],
                    start=(first_start and i == 0), stop=False,
                    skip_group_check=True,
                )

        # q^T / k^T chunks: m=0,1 -> q pairs, m=2,3 -> k pairs
        qk = [pers.tile([128, T], BF16, tag=f"qk{m}", name=f"qk{m}")
              for m in range(4)]
        # V' = [V | 1] per (tk-tile, head); ones column from bvb[:, 256:320]
        v_all = pers.tile([128, NT, NHL, HS + 1], BF16, tag="v_all",
                          name="v_all")
        nc.vector.tensor_copy(
            out=v_all[:, :, :, HS],
            in_=bvb[:, 256:320].rearrange("p (a b) -> p a b", a=NT),
        )
        # y^T chunks (normalized attention output), pair-stacked
        yt = [pers.tile([128, T], BF16, tag=f"yt{p}", name=f"yt{p}")
              for p in range(2)]

        # ---- QKV projection: qk[m] = (x @ wqk[:, m-chunk])^T + bias.
        # mq (po pool) and mk (ps pool) are emitted interleaved per x
        # chunk so the PE keeps pace with the x DMA stream. ----
        def qk_proj_pair(mq, mk, warmups=False):
            pq = [po.tile([128, TCH], F32, tag="po", name=f"pst{mq}_{i}")
                  for i in range(4)]
            wide = [ps.tile([128, 1024], F32, tag="st", name=f"pw{mk}_{i}")
                    for i in range(2)]
            pk = [wide[i // 2][:, TCH * (i % 2):TCH * (i % 2 + 1)]
                  for i in range(4)]
            for c in range(8):
                if warmups:
                    warm(pq[0][:], n=(6 if c == 0 else 0),
                         first_start=(c == 0))
                for m, pst in ((mq, pq), (mk, pk)):
                    lhsT = wqk[c][:, 128 * m:128 * (m + 1)]
                    for i in range(4):
                        nc.tensor.matmul(
                            pst[i][:],
                            lhsT,
                            xt[c][:, TCH * i:TCH * (i + 1)],
                            start=(c == 0),
                            stop=(c == 7),
                        )
            for m, pst in ((mq, pq), (mk, pk)):
                for i in range(4):
                    if i % 2 == 0:
                        nc.vector.tensor_scalar_add(
                            out=qk[m][:, TCH * i:TCH * (i + 1)],
                            in0=pst[i][:],
                            scalar1=bqk[:, m:m + 1],
                        )
                    else:
                        nc.scalar.activation(
                            out=qk[m][:, TCH * i:TCH * (i + 1)],
                            in_=pst[i][:],
                            func=AF.Identity,
                            bias=bqk[:, m:m + 1],
                            scale=1.0,
                        )

        # ---- V tile t in natural layout (+bias) ----
        def v_tile(t):
            vp = po.tile([128, 256], F32, tag="po", name=f"vp{t}")
            for c in range(8):
                nc.tensor.matmul(
                    vp[:],
                    xt[c][:, 128 * t:128 * (t + 1)],
                    wv[c][:],
                    start=(c == 0),
                    stop=(c == 7),
                )
            nc.vector.tensor_add(
                out=v_all[:, t, :, 0:HS],
                in0=vp[:].rearrange("p (h d) -> p h d", h=NHL),
                in1=bvb[:, 0:256].rearrange("p (h d) -> p h d", h=NHL),
            )

        # ---- fill queue: self-contained ~0.4-0.9us PE work units pumped
        # one per attention step, so the in-order PE queue always has
        # ready work behind the exp-gated AV matmuls ----
        fill_q = []

        def pump(k=1):
            for _ in range(k):
                if fill_q:
                    fill_q.pop(0)()

        # one 256-col slice of a late q/k projection chunk m: full c
        # accumulation in a single po slot
        def qk_slice(m, i):
            pst = po.tile([128, 256], F32, tag="po", name=f"qs{m}_{i}")
            for c in range(8):
                nc.tensor.matmul(
                    pst[:],
                    wqk[c][:, 128 * m:128 * (m + 1)],
                    xt[c][:, 256 * i:256 * (i + 1)],
                    start=(c == 0),
                    stop=(c == 7),
                )
            nc.vector.tensor_scalar_add(
                out=qk[m][:, 256 * i:256 * (i + 1)],
                in0=pst[:],
                scalar1=bqk[:, m:m + 1],
            )

        # attention-side SBUF pools
        ptp = tc.alloc_tile_pool(name="pt", bufs=36)
        lrp = tc.alloc_tile_pool(name="lrec", bufs=6)
        ynp = tc.alloc_tile_pool(name="ynp", bufs=40)
        ostp = tc.alloc_tile_pool(name="ost", bufs=4)
        # y_norm2[pair][j]: [128 tq, 2 h2, 64 hs] bf16, filled by the two
        # h2 phases of a pair, then PE-transposed into yt[pair]
        yn = [[None] * NT for _ in range(2)]

        def drain_j(pair, h2, j, av):
            # l sits at column 64 of the flipped AV output: per-partition,
            # so normalization is a reciprocal + tensor_scalar multiply
            if h2 == 0:
                yn[pair][j] = ynp.tile([128, 2, HS], BF16, tag="yn",
                                       name=f"yn{pair}{j}")
            lr = lrp.tile([128, 1], F32, tag="lr", name=f"lr{pair}{h2}{j}")
            nc.vector.reciprocal(out=lr[:], in_=av[:, HS:HS + 1])
            nc.vector.tensor_scalar_mul(
                out=yn[pair][j][:, h2, :],
                in0=av[:, 0:HS],
                scalar1=lr[:],
            )
            if h2 == 1:
                pending_tp.append((pair, j))

        pending_tp = []

        def flush_tp():
            # transpose [128,128] into y^T via the PE (identity moving
            # operand), staged through PSUM; deferred a step so the PE
            # never parks waiting on the drain's DVE chain
            while pending_tp:
                pair, j = pending_tp.pop(0)
                tp = po.tile([128, 128], BF16, tag="po", name=f"tp{pair}{j}")
                nc.tensor.matmul(
                    tp[:],
                    yn[pair][j].rearrange("p a b -> p (a b)"),
                    idn[:],
                    start=True, stop=True, is_transpose=True,
                )
                nc.vector.tensor_copy(
                    out=yt[pair][:, 128 * j:128 * (j + 1)], in_=tp[:])

        # ---- c_proj partial tile t: out rows [128t, 128t+128) ----
        stg_live = {}

        def proj_half(t, oc, tail=False):
            if oc == 0:
                stg_live[t] = ostp.tile([128, 1024], BF16, tag="stg",
                                        name=f"stg{t}")
            stg = stg_live[t]
            pp = po.tile([128, TCH], F32, tag="po", name=f"pp{t}{oc}")
            for p2 in range(2):
                nc.tensor.matmul(
                    pp[:],
                    yt[p2][:, 128 * t:128 * (t + 1)],
                    wp[p2][:, TCH * oc:TCH * (oc + 1)],
                    start=(p2 == 0),
                    stop=(p2 == 1),
                )
            if tail and oc == 1:
                # Act is idle at the tail; split copies across engines
                nc.scalar.copy(out=stg[:, TCH:2 * TCH], in_=pp[:])
            else:
                nc.vector.tensor_copy(
                    out=stg[:, TCH * oc:TCH * (oc + 1)], in_=pp[:])
            if oc == 1:
                nc.sync.dma_start(out=out_d[128 * t:128 * (t + 1), :],
                                  in_=stg[:])
                del stg_live[t]

        def proj_tile(t, tail=False):
            proj_half(t, 0, tail)
            proj_half(t, 1, tail)

        # ---- attention phase for one (pair, half, h2): head-serial so
        # only 2 opr banks are live, leaving po slots for interleaved
        # proj/V work. extra(t) emits interleaved PE work after step t. ----
        def attn(pair, half, h2, extra=None):
            t_end = 8 * (half + 1)
            h = 2 * pair + h2
            pb = 64 * h2
            pts = {}

            def emit_S(t):
                # row-packed K=64 matmul: head h2 lives at partitions
                # 64*h2..64*h2+64 of the qk chunks; moving dim trimmed
                # to the causal boundary (no small-N penalty in bf16).
                st = ps.tile([128, 1024], F32, tag="st",
                             name=f"st{pair}{half}{t}{h2}")
                for cc in range(2):
                    cg = 2 * half + cc
                    if cg < t // 4:
                        continue
                    sub0 = max(0, 128 * t - TCH * cg)
                    nc.tensor.matmul(
                        st[:, TCH * cc + sub0:TCH * (cc + 1)],
                        qk[2 + pair][pb:pb + 64, 128 * t:128 * (t + 1)],
                        qk[pair][pb:pb + 64, TCH * cg + sub0:TCH * (cg + 1)],
                        start=True,
                        stop=True,
                    )
                return st

            # software-pipelined emission: the PE queue is in-order, so
            # S(t+1) and any interleaved filler must be emitted BEFORE
            # AV(t), which parks waiting on exp(t).
            st = emit_S(0)
            for t in range(t_end):
                rel = max(128 * t, 1024 * half) - 1024 * half
                pt = ptp.tile([128, 1024], BF16, tag="pt",
                              name=f"pt{pair}{half}{t}{h2}")
                nc.scalar.activation(
                    out=pt[:, rel:1024], in_=st[:, rel:1024], func=AF.Exp
                )
                if t + 1 < t_end:
                    st = emit_S(t + 1)
                pump(1)
                flush_tp()
                if t // 8 == half:
                    # zero strict-lower triangle (tk > tq) of diag block
                    nc.vector.tensor_mul(
                        out=pt[:, rel:rel + 128],
                        in0=pt[:, rel:rel + 128],
                        in1=msk[:],
                    )
                pts[t] = pt
                # flipped AV, j-major burst: once exp(t) lands, the output
                # tile for tq-tile j == t is fully determined; accumulate
                # it over all pt(t' <= t) in one go. Stationary P^T 128-col
                # slice, moving V' [128, 65] -> out [tq, 65]: 65-cycle
                # matmuls, and l lands as a per-partition column.
                jj = t - 8 * half
                if jj >= 0:
                    avt = po.tile([128, HS + 1], F32, tag="po",
                                  name=f"av{pair}{half}{h2}{jj}")
                    for tp_ in range(t + 1):
                        nc.tensor.matmul(
                            avt[:],
                            pts[tp_][:, 128 * jj:128 * (jj + 1)],
                            v_all[:, tp_, h, :],
                            start=(tp_ == 0),
                            stop=(tp_ == t),
                        )
                    drain_j(pair, h2, t, avt)
                if extra is not None:
                    extra(t)
            flush_tp()

        def attn2(pair, half, extra=None):
            t_end = 8 * (half + 1)
            h0 = 2 * pair
            pts = {}

            def emit_S2(t, h2):
                st = ps.tile([128, 1024], F32, tag="st",
                             name=f"s2{pair}{half}{t}{h2}")
                pb = 64 * h2
                for cc in range(2):
                    cg = 2 * half + cc
                    if cg < t // 4:
                        continue
                    sub0 = max(0, 128 * t - TCH * cg)
                    nc.tensor.matmul(
                        st[:, TCH * cc + sub0:TCH * (cc + 1)],
                        qk[2 + pair][pb:pb + 64, 128 * t:128 * (t + 1)],
                        qk[pair][pb:pb + 64, TCH * cg + sub0:TCH * (cg + 1)],
                        start=True,
                        stop=True,
                    )
                return st

            st2 = [emit_S2(0, 0), emit_S2(0, 1)]
            for t in range(t_end):
                rel = max(128 * t, 1024 * half) - 1024 * half
                for h2 in range(2):
                    pt = ptp.tile([128, 1024], BF16, tag="pt",
                                  name=f"p2{pair}{half}{t}{h2}")
                    nc.scalar.activation(
                        out=pt[:, rel:1024], in_=st2[h2][:, rel:1024],
                        func=AF.Exp,
                    )
                    if t + 1 < t_end:
                        st2[h2] = emit_S2(t + 1, h2)
                    pump(1)
                    if t // 8 == half:
                        nc.vector.tensor_mul(
                            out=pt[:, rel:rel + 128],
                            in0=pt[:, rel:rel + 128],
                            in1=msk[:],
                        )
                    pts[(t, h2)] = pt
                    jj = t - 8 * half
                    if jj >= 0:
                        avt = po.tile([128, HS + 1], F32, tag="po",
                                      name=f"a2{pair}{half}{h2}{jj}")
                        for tp_ in range(t + 1):
                            nc.tensor.matmul(
                                avt[:],
                                pts[(tp_, h2)][:, 128 * jj:128 * (jj + 1)],
                                v_all[:, tp_, h0 + h2, :],
                                start=(tp_ == 0),
                                stop=(tp_ == t),
                            )
                        drain_j(pair, h2, t, avt)
                    flush_tp()
                if extra is not None:
                    extra(t)

        # -------- schedule --------
        # Every attention phase is exp(Act)-bound; all remaining PE work
        # (V tiles, pair-1 q/k quarters, c_proj tiles) is interleaved into
        # those phases the moment its dependencies allow.
        def mk_extra(fns, at):
            sched = dict(zip(at, fns))
            return lambda t: sched[t]() if t in sched else None

        qk_proj_pair(0, 2, warmups=True)  # pair-0 q/k, paced by the x stream
        for t in range(3):
            v_tile(t)

        def mk_extra(fns, at):
            sched = dict(zip(at, fns))
            return lambda t: sched[t]() if t in sched else None

        # pair-0 half-0 attention; V tiles 3..15 stream through its exp
        # gaps three steps ahead of their first AV use
        attn(0, 0, 0, extra=lambda t: v_tile(3 + t) if t < 8 else None)
        attn(0, 0, 1, extra=lambda t: v_tile(11 + t) if t < 5 else None)

        # pair-0 half-1 attention; pair-1 q/k projection slices ride in
        # its exp gaps
        attn(0, 1, 0, extra=mk_extra(
            [lambda i=i: qk_slice(1, i) for i in range(4)] +
            [lambda i=i: qk_slice(3, i) for i in range(4)],
            [1, 3, 5, 7, 9, 11, 13, 15]))
        attn(0, 1, 1, extra=mk_extra(
            [lambda i=i: qk_slice(1, i) for i in range(4, 8)] +
            [lambda i=i: qk_slice(3, i) for i in range(4, 8)],
            [1, 3, 5, 7, 9, 11, 13, 15]))

        attn(1, 0, 0)
        # c_proj half-tiles interleave as soon as their yt columns are
        # complete (tiles 0-3 need only the first 512 tq columns)
        attn(1, 0, 1, extra=mk_extra(
            [lambda t=t, oc=oc: proj_half(t, oc, oc == 1)
             for t in (0, 1) for oc in (0, 1)], [4, 5, 6, 7]))
        p2sched = {k: [lambda t=2 + k, oc=oc: proj_half(t, oc)
                       for oc in (0, 1)] for k in range(6)}
        for k in range(6):
            # late merged-phase steps: exp is short there, so the oc=1
            # staging copy goes to the otherwise-idle Act engine
            p2sched[10 + k] = [lambda t=8 + k, oc=oc: proj_half(t, oc, True)
                               for oc in (0, 1)]
        attn2(1, 1, extra=lambda t: [f() for f in p2sched.get(t, [])])
        proj_tile(14, tail=True)
        proj_tile(15, tail=True)
        ostp.release()
        ynp.release()
        lrp.release()
        ptp.release()
        xp.release()


_PROG = None


def _get_program():
    global _PROG
    if _PROG is None:
        _PROG = build_program()
    return _PROG


def _bf(a):
    return np.ascontiguousarray(np.asarray(a, dtype=ml_dtypes.bfloat16))


def make_in_maps(x, w_attn, b_attn, w_proj, b_proj):
    x = np.asarray(x, dtype=np.float32)
    w_attn = np.asarray(w_attn, dtype=np.float32)
    b_attn = np.asarray(b_attn, dtype=np.float32)
    w_proj = np.asarray(w_proj, dtype=np.float32)
    s = 1.0 / np.sqrt(HS)
    wq, wk, wv = w_attn[:, 0:C], w_attn[:, C:2 * C], w_attn[:, 2 * C:3 * C]
    bq, bk, bv = b_attn[0:C], b_attn[C:2 * C], b_attn[2 * C:3 * C]
    # upper-triangular-inclusive causal mask for the S^T diagonal block
    msk = np.triu(np.ones((128, 128), dtype=np.float32))
    in_maps = []
    for core in range(NCORES):
        b, g = divmod(core, 4)
        cs = slice(256 * g, 256 * (g + 1))
        bqk_ = np.concatenate([bq[cs] * s, bk[cs]]).reshape(4, 128).T.copy()
        in_maps.append({
            "xT": _bf(x[b].T),
            "wqk": _bf(np.concatenate([wq[:, cs] * s, wk[:, cs]], axis=1)),
            "wv": _bf(wv[:, cs]),
            "wp": _bf(w_proj[cs, :]),
            "bqk": np.ascontiguousarray(bqk_),
            "bvb": _bf(np.concatenate([
                np.broadcast_to(bv[cs][None, :], (128, 256)),
                np.ones((128, 64), dtype=np.float32)], axis=1)),
            "msk": _bf(msk),
            "idn": _bf(np.eye(128, dtype=np.float32)),
        })
    return in_maps


def gather_output(results, b_proj):
    b_proj = np.asarray(b_proj, dtype=np.float32)
    out = np.empty((B, T, C), dtype=np.float32)
    for b in range(B):
        acc = results[4 * b]["out"].astype(np.float32)
        for g in range(1, 4):
            acc = acc + results[4 * b + g]["out"].astype(np.float32)
        out[b] = acc + b_proj[None, :]
    return out


def kernel(x, w_attn, b_attn, w_proj, b_proj):
    nc = _get_program()
    in_maps = make_in_maps(x, w_attn, b_attn, w_proj, b_proj)
    res = run_bass_kernel_spmd(nc, in_maps, core_ids=list(range(NCORES)))
    return gather_output(res.results, b_proj)


# revision 36
# speedup vs baseline: 1.0014x; 1.0014x over previous
"""Causal self-attention on 8 trn2 NeuronCores (bf16 datapath).

Problem: B=2, T=2048, C=1024, 16 heads of 64. Sharding: core = 4*b + g
(b = batch, g = head-group of 4 heads). Each core computes QKV projection
for its 4 heads, causal attention, and a partial c_proj (its 256 rows of
w_proj). Host sums the 4 partials per batch (the "all-reduce") + b_proj.

All SBUF operands are bf16 (PSUM accumulation stays fp32), which halves
input DMA bytes and removes the fp32r small-N matmul penalty, so the
S / AV matmuls can be trimmed to the causal boundary at 128-column
granularity.

Schedule: weights stream on the Activation HWDGE queue (wqk first),
x^T on the sync HWDGE queue; the pair-0 q/k projections consume x chunks
as they land, with zero-matmul warmups pinning the PE p-state ramp
through the DMA-paced stretch. Attention runs head-serial (one (pair,
half, h2) phase at a time), software-pipelined: S(t+1) and interleaved
filler (V tiles, pair-1 q/k quarter-projections, c_proj tiles) are
emitted before the AV work that parks on exp(t). AV runs flipped
(stationary P^T 128-col slice, moving V' -> out [tq, 65]) so each AV
matmul costs 65 PE cycles instead of ~512, and the softmax denominator
lands as a per-partition column: normalization is reciprocal +
tensor_scalar on DVE, then a PE transpose (identity moving operand)
rebuilds y^T for c_proj.

Per-core layouts (partition dim first):
  xT      (1024, 2048)  x[b]^T bf16; SBUF as 8 chunks (128, 2048)
  q^T/k^T (256, 2048)   bf16, 2 chunks each; chunk p = heads 2p, 2p+1;
                        1/sqrt(hs) folded into Wq,bq
  V'      (2048, 4, 65) bf16, natural + ones column (fused softmax
                        denominator: AV matmul emits [O^T; l] with M=65)
  S^T     (tk, tq) PSUM fp32; exp on ScalarE (no max subtraction: logits
                        ~N(0,1), exp cannot overflow); P^T bf16 in SBUF
  diag    strict-lower triangle of the diagonal 128x128 block is zeroed
                        by a bf16 upper-tri mask multiply on the DVE
  y^T     (256, 2048)   bf16 normalized attention out
  out     (2048, 1024)  bf16 partial y@w_proj; host sums in fp32
"""

import numpy as np
import ml_dtypes

import concourse.tile as tile
from concourse import bacc, mybir
from concourse.bass_utils import run_bass_kernel_spmd

B, T, C = 2, 2048, 1024
HS = 64
NCORES = 8
NHL = 4            # heads per core
TCH = 512          # tq / projection T chunk
NT = T // 128      # 16 tk tiles
F32 = mybir.dt.float32
BF16 = mybir.dt.bfloat16


def build_program():
    nc = bacc.Bacc("TRN2", target_bir_lowering=False, debug=False)

    xT_d = nc.dram_tensor("xT", [C, T], BF16, kind="ExternalInput").ap()
    wqk_d = nc.dram_tensor("wqk", [C, 512], BF16, kind="ExternalInput").ap()
    wv_d = nc.dram_tensor("wv", [C, 256], BF16, kind="ExternalInput").ap()
    wp_d = nc.dram_tensor("wp", [256, 1024], BF16, kind="ExternalInput").ap()
    bqk_d = nc.dram_tensor("bqk", [128, 4], F32, kind="ExternalInput").ap()
    bvb_d = nc.dram_tensor("bvb", [128, 320], BF16, kind="ExternalInput").ap()
    msk_d = nc.dram_tensor("msk", [128, 128], BF16, kind="ExternalInput").ap()
    idn_d = nc.dram_tensor("idn", [128, 128], BF16, kind="ExternalInput").ap()
    out_d = nc.dram_tensor("out", [T, C], BF16, kind="ExternalOutput").ap()

    with tile.TileContext(nc) as tc:
        _kernel(tc, out_d, xT_d, wqk_d, wv_d, wp_d, bqk_d, bvb_d, msk_d, idn_d)
    nc.compile()
    return nc


def _kernel(tc, out_d, xT_d, wqk_d, wv_d, wp_d, bqk_d, bvb_d, msk_d, idn_d):
    nc = tc.nc
    AF = mybir.ActivationFunctionType

    with (
        tc.tile_pool(name="persist", bufs=1) as pers,
        tc.tile_pool(name="ps", bufs=2, space="PSUM") as ps,
        tc.tile_pool(name="po", bufs=4, space="PSUM") as po,
    ):
        # Weights on the Activation HWDGE queue (bqk + wqk lead so the
        # first q-projection matmul only waits for wqk[0] + xt[0]); x^T
        # streams on the sync HWDGE queue in parallel.
        xp = tc.alloc_tile_pool(name="xp", bufs=1)
        xT3 = xT_d.rearrange("(c p) t -> c p t", p=128)
        wqk3 = wqk_d.rearrange("(c p) m -> c p m", p=128)
        wv3 = wv_d.rearrange("(c p) m -> c p m", p=128)
        wp3 = wp_d.rearrange("(c p) m -> c p m", p=128)
        xt, wqk, wv = [], [], []
        for c in range(8):
            w_ = pers.tile([128, 512], BF16, tag=f"wqk{c}", name=f"wqk{c}")
            nc.scalar.dma_start(out=w_, in_=wqk3[c])
            wqk.append(w_)
            t_ = xp.tile([128, T], BF16, tag=f"xt{c}", name=f"xt{c}")
            nc.sync.dma_start(out=t_, in_=xT3[c])
            xt.append(t_)
        bqk = pers.tile([128, 4], F32, tag="bqk")
        nc.scalar.dma_start(out=bqk, in_=bqk_d)
        for c in range(8):
            t_ = pers.tile([128, 256], BF16, tag=f"wv{c}", name=f"wv{c}")
            nc.scalar.dma_start(out=t_, in_=wv3[c])
            wv.append(t_)
        bvb = pers.tile([128, 320], BF16, tag="bvb")
        nc.scalar.dma_start(out=bvb, in_=bvb_d)
        msk = pers.tile([128, 128], BF16, tag="msk")
        nc.scalar.dma_start(out=msk, in_=msk_d)
        idn = pers.tile([128, 128], BF16, tag="idn")
        nc.scalar.dma_start(out=idn, in_=idn_d)
        wp = []
        for c in range(2):
            t_ = pers.tile([128, 1024], BF16, tag=f"wp{c}", name=f"wp{c}")
            nc.scalar.dma_start(out=t_, in_=wp3[c])
            wp.append(t_)

        # zero operands for PE-warmup matmuls: accumulate +0 into live
        # PSUM tiles purely to keep the tensor engine's p-state ramp hot
        # across DMA-paced stretches (an idle PE drops back to half clock
        # for the next 3us).
        zs = pers.tile([64, 128], BF16, tag="zs")
        nc.gpsimd.memset(zs[:], 0)
        zs2 = pers.tile([64, 512], BF16, tag="zs2")
        nc.gpsimd.memset(zs2[:], 0)

        def warm(pst, n=1, cols=512, first_start=False):
            for i in range(n):
                nc.tensor.matmul(
                    pst[0:128, 0:cols], zs[:], zs2[:, 0:cols],
                    start=(first_start and i == 0), stop=False,
                    skip_group_check=True,
                )

        # q^T / k^T chunks: m=0,1 -> q pairs, m=2,3 -> k pairs
        qk = [pers.tile([128, T], BF16, tag=f"qk{m}", name=f"qk{m}")
              for m in range(4)]
        # V' = [V | 1] per (tk-tile, head); ones column from bvb[:, 256:320]
        v_all = pers.tile([128, NT, NHL, HS + 1], BF16, tag="v_all",
                          name="v_all")
        nc.vector.tensor_copy(
            out=v_all[:, :, :, HS],
            in_=bvb[:, 256:320].rearrange("p (a b) -> p a b", a=NT),
        )
        # y^T chunks (normalized attention output), pair-stacked
        yt = [pers.tile([128, T], BF16, tag=f"yt{p}", name=f"yt{p}")
              for p in range(2)]

        # ---- QKV projection: qk[m] = (x @ wqk[:, m-chunk])^T + bias.
        # mq (po pool) and mk (ps pool) are emitted interleaved per x
        # chunk so the PE keeps pace with the x DMA stream. ----
        def qk_proj_pair(mq, mk, warmups=False):
            pq = [po.tile([128, TCH], F32, tag="po", name=f"pst{mq}_{i}")
                  for i in range(4)]
            wide = [ps.tile([128, 1024], F32, tag="st", name=f"pw{mk}_{i}")
                    for i in range(2)]
            pk = [wide[i // 2][:, TCH * (i % 2):TCH * (i % 2 + 1)]
                  for i in range(4)]
            for c in range(8):
                if warmups:
                    warm(pq[0][:], n=(6 if c == 0 else 0),
                         first_start=(c == 0))
                for m, pst in ((mq, pq), (mk, pk)):
                    lhsT = wqk[c][:, 128 * m:128 * (m + 1)]
                    for i in range(4):
                        nc.tensor.matmul(
                            pst[i][:],
                            lhsT,
                            xt[c][:, TCH * i:TCH * (i + 1)],
                            start=(c == 0),
                            stop=(c == 7),
                        )
            for m, pst in ((mq, pq), (mk, pk)):
                for i in range(4):
                    if i % 2 == 0:
                        nc.vector.tensor_scalar_add(
                            out=qk[m][:, TCH * i:TCH * (i + 1)],
                            in0=pst[i][:],
                            scalar1=bqk[:, m:m + 1],
                        )
                    else:
                        nc.scalar.activation(
                            out=qk[m][:, TCH * i:TCH * (i + 1)],
                            in_=pst[i][:],
                            func=AF.Identity,
                            bias=bqk[:, m:m + 1],
                            scale=1.0,
                        )

        # ---- V tile t in natural layout (+bias) ----
        def v_tile(t):
            vp = po.tile([128, 256], F32, tag="po", name=f"vp{t}")
            for c in range(8):
                nc.tensor.matmul(
                    vp[:],
                    xt[c][:, 128 * t:128 * (t + 1)],
                    wv[c][:],
                    start=(c == 0),
                    stop=(c == 7),
                )
            nc.vector.tensor_add(
                out=v_all[:, t, :, 0:HS],
                in0=vp[:].rearrange("p (h d) -> p h d", h=NHL),
                in1=bvb[:, 0:256].rearrange("p (h d) -> p h d", h=NHL),
            )

        # ---- fill queue: self-contained ~0.4-0.9us PE work units pumped
        # one per attention step, so the in-order PE queue always has
        # ready work behind the exp-gated AV matmuls ----
        fill_q = []

        def pump(k=1):
            for _ in range(k):
                if fill_q:
                    fill_q.pop(0)()

        # one 256-col slice of a late q/k projection chunk m: full c
        # accumulation in a single po slot
        def qk_slice(m, i):
            pst = po.tile([128, 256], F32, tag="po", name=f"qs{m}_{i}")
            for c in range(8):
                nc.tensor.matmul(
                    pst[:],
                    wqk[c][:, 128 * m:128 * (m + 1)],
                    xt[c][:, 256 * i:256 * (i + 1)],
                    start=(c == 0),
                    stop=(c == 7),
                )
            nc.vector.tensor_scalar_add(
                out=qk[m][:, 256 * i:256 * (i + 1)],
                in0=pst[:],
                scalar1=bqk[:, m:m + 1],
            )

        # attention-side SBUF pools
        ptp = tc.alloc_tile_pool(name="pt", bufs=36)
        lrp = tc.alloc_tile_pool(name="lrec", bufs=6)
        ynp = tc.alloc_tile_pool(name="ynp", bufs=40)
        ostp = tc.alloc_tile_pool(name="ost", bufs=4)
        # y_norm2[pair][j]: [128 tq, 2 h2, 64 hs] bf16, filled by the two
        # h2 phases of a pair, then PE-transposed into yt[pair]
        yn = [[None] * NT for _ in range(2)]

        def drain_j(pair, h2, j, av):
            # l sits at column 64 of the flipped AV output: per-partition,
            # so normalization is a reciprocal + tensor_scalar multiply
            if h2 == 0:
                yn[pair][j] = ynp.tile([128, 2, HS], BF16, tag="yn",
                                       name=f"yn{pair}{j}")
            lr = lrp.tile([128, 1], F32, tag="lr", name=f"lr{pair}{h2}{j}")
            nc.vector.reciprocal(out=lr[:], in_=av[:, HS:HS + 1])
            nc.vector.tensor_scalar_mul(
                out=yn[pair][j][:, h2, :],
                in0=av[:, 0:HS],
                scalar1=lr[:],
            )
            if h2 == 1:
                pending_tp.append((pair, j))

        pending_tp = []

        def flush_tp():
            # transpose [128,128] into y^T via the PE (identity moving
            # operand), staged through PSUM; deferred a step so the PE
            # never parks waiting on the drain's DVE chain
            while pending_tp:
                pair, j = pending_tp.pop(0)
                tp = po.tile([128, 128], BF16, tag="po", name=f"tp{pair}{j}")
                nc.tensor.matmul(
                    tp[:],
                    yn[pair][j].rearrange("p a b -> p (a b)"),
                    idn[:],
                    start=True, stop=True, is_transpose=True,
                )
                nc.vector.tensor_copy(
                    out=yt[pair][:, 128 * j:128 * (j + 1)], in_=tp[:])

        # ---- c_proj partial tile t: out rows [128t, 128t+128) ----
        stg_live = {}

        def proj_half(t, oc, tail=False):
            if oc == 0:
                stg_live[t] = ostp.tile([128, 1024], BF16, tag="stg",
                                        name=f"stg{t}")
            stg = stg_live[t]
            pp = po.tile([128, TCH], F32, tag="po", name=f"pp{t}{oc}")
            for p2 in range(2):
                nc.tensor.matmul(
                    pp[:],
                    yt[p2][:, 128 * t:128 * (t + 1)],
                    wp[p2][:, TCH * oc:TCH * (oc + 1)],
                    start=(p2 == 0),
                    stop=(p2 == 1),
                )
            if tail and oc == 1:
                # Act is idle at the tail; split copies across engines
                nc.scalar.copy(out=stg[:, TCH:2 * TCH], in_=pp[:])
            else:
                nc.vector.tensor_copy(
                    out=stg[:, TCH * oc:TCH * (oc + 1)], in_=pp[:])
            if oc == 1:
                nc.sync.dma_start(out=out_d[128 * t:128 * (t + 1), :],
                                  in_=stg[:])
                del stg_live[t]

        def proj_tile(t, tail=False):
            proj_half(t, 0, tail)
            proj_half(t, 1, tail)

        # ---- attention phase for one (pair, half, h2): head-serial so
        # only 2 opr banks are live, leaving po slots for interleaved
        # proj/V work. extra(t) emits interleaved PE work after step t. ----
        def attn(pair, half, h2, extra=None):
            t_end = 8 * (half + 1)
            h = 2 * pair + h2
            pb = 64 * h2
            pts = {}

            def emit_S(t):
                # row-packed K=64 matmul: head h2 lives at partitions
                # 64*h2..64*h2+64 of the qk chunks; moving dim trimmed
                # to the causal boundary (no small-N penalty in bf16).
                st = ps.tile([128, 1024], F32, tag="st",
                             name=f"st{pair}{half}{t}{h2}")
                for cc in range(2):
                    cg = 2 * half + cc
                    if cg < t // 4:
                        continue
                    sub0 = max(0, 128 * t - TCH * cg)
                    nc.tensor.matmul(
                        st[:, TCH * cc + sub0:TCH * (cc + 1)],
                        qk[2 + pair][pb:pb + 64, 128 * t:128 * (t + 1)],
                        qk[pair][pb:pb + 64, TCH * cg + sub0:TCH * (cg + 1)],
                        start=True,
                        stop=True,
                    )
                return st

            # software-pipelined emission: the PE queue is in-order, so
            # S(t+1) and any interleaved filler must be emitted BEFORE
            # AV(t), which parks waiting on exp(t).
            st = emit_S(0)
            for t in range(t_end):
                rel = max(128 * t, 1024 * half) - 1024 * half
                pt = ptp.tile([128, 1024], BF16, tag="pt",
                              name=f"pt{pair}{half}{t}{h2}")
                nc.scalar.activation(
                    out=pt[:, rel:1024], in_=st[:, rel:1024], func=AF.Exp
                )
                if t + 1 < t_end:
                    st = emit_S(t + 1)
                pump(1)
                flush_tp()
                if t // 8 == half:
                    # zero strict-lower triangle (tk > tq) of diag block
                    nc.vector.tensor_mul(
                        out=pt[:, rel:rel + 128],
                        in0=pt[:, rel:rel + 128],
                        in1=msk[:],
                    )
                pts[t] = pt
                # flipped AV, j-major burst: once exp(t) lands, the output
                # tile for tq-tile j == t is fully determined; accumulate
                # it over all pt(t' <= t) in one go. Stationary P^T 128-col
                # slice, moving V' [128, 65] -> out [tq, 65]: 65-cycle
                # matmuls, and l lands as a per-partition column.
                jj = t - 8 * half
                if jj >= 0:
                    avt = po.tile([128, HS + 1], F32, tag="po",
                                  name=f"av{pair}{half}{h2}{jj}")
                    for tp_ in range(t + 1):
                        nc.tensor.matmul(
                            avt[:],
                            pts[tp_][:, 128 * jj:128 * (jj + 1)],
                            v_all[:, tp_, h, :],
                            start=(tp_ == 0),
                            stop=(tp_ == t),
                        )
                    drain_j(pair, h2, t, avt)
                if extra is not None:
                    extra(t)
            flush_tp()

        def attn2(pair, half, extra=None):
            t_end = 8 * (half + 1)
            h0 = 2 * pair
            pts = {}

            def emit_S2(t, h2):
                st = ps.tile([128, 1024], F32, tag="st",
                             name=f"s2{pair}{half}{t}{h2}")
                pb = 64 * h2
                for cc in range(2):
                    cg = 2 * half + cc
                    if cg < t // 4:
                        continue
                    sub0 = max(0, 128 * t - TCH * cg)
                    nc.tensor.matmul(
                        st[:, TCH * cc + sub0:TCH * (cc + 1)],
                        qk[2 + pair][pb:pb + 64, 128 * t:128 * (t + 1)],
                        qk[pair][pb:pb + 64, TCH * cg + sub0:TCH * (cg + 1)],
                        start=True,
                        stop=True,
                    )
                return st

            st2 = [emit_S2(0, 0), emit_S2(0, 1)]
            for t in range(t_end):
                rel = max(128 * t, 1024 * half) - 1024 * half
                for h2 in range(2):
                    pt = ptp.tile([128, 1024], BF16, tag="pt",
                                  name=f"p2{pair}{half}{t}{h2}")
                    nc.scalar.activation(
                        out=pt[:, rel:1024], in_=st2[h2][:, rel:1024],
                        func=AF.Exp,
                    )
                    if t + 1 < t_end:
                        st2[h2] = emit_S2(t + 1, h2)
                    pump(1)
                    if t // 8 == half:
                        nc.vector.tensor_mul(
                            out=pt[:, rel:rel + 128],
                            in0=pt[:, rel:rel + 128],
                            in1=msk[:],
                        )
                    pts[(t, h2)] = pt
                    jj = t - 8 * half
                    if jj >= 0:
                        avt = po.tile([128, HS + 1], F32, tag="po",
                                      name=f"a2{pair}{half}{h2}{jj}")
                        for tp_ in range(t + 1):
                            nc.tensor.matmul(
                                avt[:],
                                pts[(tp_, h2)][:, 128 * jj:128 * (jj + 1)],
                                v_all[:, tp_, h0 + h2, :],
                                start=(tp_ == 0),
                                stop=(tp_ == t),
                            )
                        drain_j(pair, h2, t, avt)
                    flush_tp()
                if extra is not None:
                    extra(t)

        # -------- schedule --------
        # Every attention phase is exp(Act)-bound; all remaining PE work
        # (V tiles, pair-1 q/k quarters, c_proj tiles) is interleaved into
        # those phases the moment its dependencies allow.
        def mk_extra(fns, at):
            sched = dict(zip(at, fns))
            return lambda t: sched[t]() if t in sched else None

        qk_proj_pair(0, 2, warmups=True)  # pair-0 q/k, paced by the x stream
        for t in range(3):
            v_tile(t)

        def mk_extra(fns, at):
            sched = dict(zip(at, fns))
            return lambda t: sched[t]() if t in sched else None

        # pair-0 half-0 attention; V tiles 3..15 stream through its exp
        # gaps three steps ahead of their first AV use
        attn(0, 0, 0, extra=lambda t: v_tile(3 + t) if t < 8 else None)
        attn(0, 0, 1, extra=lambda t: v_tile(11 + t) if t < 5 else None)

        # pair-0 half-1 attention; pair-1 q/k projection slices ride in
        # its exp gaps
        attn(0, 1, 0, extra=mk_extra(
            [lambda i=i: qk_slice(1, i) for i in range(4)] +
            [lambda i=i: qk_slice(3, i) for i in range(4)],
            [1, 3, 5, 7, 9, 11, 13, 15]))
        attn(0, 1, 1, extra=mk_extra(
            [lambda i=i: qk_slice(1, i) for i in range(4, 8)] +
            [lambda i=i: qk_slice(3, i) for i in range(4, 8)],
            [1, 3, 5, 7, 9, 11, 13, 15]))

        attn(1, 0, 0)
        # c_proj half-tiles interleave as soon as their yt columns are
        # complete (tiles 0-3 need only the first 512 tq columns)
        attn(1, 0, 1, extra=mk_extra(
            [lambda t=t, oc=oc: proj_half(t, oc)
             for t in (0, 1) for oc in (0, 1)], [4, 5, 6, 7]))
        p2sched = {k: [lambda t=2 + k, oc=oc: proj_half(t, oc)
                       for oc in (0, 1)] for k in range(6)}
        for k in range(6):
            # late merged-phase steps: exp is short there, so the oc=1
            # staging copy goes to the otherwise-idle Act engine
            p2sched[10 + k] = [lambda t=8 + k, oc=oc: proj_half(t, oc, True)
                               for oc in (0, 1)]
        attn2(1, 1, extra=lambda t: [f() for f in p2sched.get(t, [])])
        proj_tile(14, tail=True)
        proj_tile(15, tail=True)
        ostp.release()
        ynp.release()
        lrp.release()
        ptp.release()
        xp.release()


_PROG = None


def _get_program():
    global _PROG
    if _PROG is None:
        _PROG = build_program()
    return _PROG


def _bf(a):
    return np.ascontiguousarray(np.asarray(a, dtype=ml_dtypes.bfloat16))


def make_in_maps(x, w_attn, b_attn, w_proj, b_proj):
    x = np.asarray(x, dtype=np.float32)
    w_attn = np.asarray(w_attn, dtype=np.float32)
    b_attn = np.asarray(b_attn, dtype=np.float32)
    w_proj = np.asarray(w_proj, dtype=np.float32)
    s = 1.0 / np.sqrt(HS)
    wq, wk, wv = w_attn[:, 0:C], w_attn[:, C:2 * C], w_attn[:, 2 * C:3 * C]
    bq, bk, bv = b_attn[0:C], b_attn[C:2 * C], b_attn[2 * C:3 * C]
    # upper-triangular-inclusive causal mask for the S^T diagonal block
    msk = np.triu(np.ones((128, 128), dtype=np.float32))
    in_maps = []
    for core in range(NCORES):
        b, g = divmod(core, 4)
        cs = slice(256 * g, 256 * (g + 1))
        bqk_ = np.concatenate([bq[cs] * s, bk[cs]]).reshape(4, 128).T.copy()
        in_maps.append({
            "xT": _bf(x[b].T),
            "wqk": _bf(np.concatenate([wq[:, cs] * s, wk[:, cs]], axis=1)),
            "wv": _bf(wv[:, cs]),
            "wp": _bf(w_proj[cs, :]),
            "bqk": np.ascontiguousarray(bqk_),
            "bvb": _bf(np.concatenate([
                np.broadcast_to(bv[cs][None, :], (128, 256)),
                np.ones((128, 64), dtype=np.float32)], axis=1)),
            "msk": _bf(msk),
            "idn": _bf(np.eye(128, dtype=np.float32)),
        })
    return in_maps


def gather_output(results, b_proj):
    b_proj = np.asarray(b_proj, dtype=np.float32)
    out = np.empty((B, T, C), dtype=np.float32)
    for b in range(B):
        acc = results[4 * b]["out"].astype(np.float32)
        for g in range(1, 4):
            acc = acc + results[4 * b + g]["out"].astype(np.float32)
        out[b] = acc + b_proj[None, :]
    return out


def kernel(x, w_attn, b_attn, w_proj, b_proj):
    nc = _get_program()
    in_maps = make_in_maps(x, w_attn, b_attn, w_proj, b_proj)
    res = run_bass_kernel_spmd(nc, in_maps, core_ids=list(range(NCORES)))
    return gather_output(res.results, b_proj)


# revision 37
# speedup vs baseline: 1.0148x; 1.0133x over previous
"""Causal self-attention on 8 trn2 NeuronCores (bf16 datapath).

Problem: B=2, T=2048, C=1024, 16 heads of 64. Sharding: core = 4*b + g
(b = batch, g = head-group of 4 heads). Each core computes QKV projection
for its 4 heads, causal attention, and a partial c_proj (its 256 rows of
w_proj). Host sums the 4 partials per batch (the "all-reduce") + b_proj.

All SBUF operands are bf16 (PSUM accumulation stays fp32), which halves
input DMA bytes and removes the fp32r small-N matmul penalty, so the
S / AV matmuls can be trimmed to the causal boundary at 128-column
granularity.

Schedule: weights stream on the Activation HWDGE queue (wqk first),
x^T on the sync HWDGE queue; the pair-0 q/k projections consume x chunks
as they land, with zero-matmul warmups pinning the PE p-state ramp
through the DMA-paced stretch. Attention runs head-serial (one (pair,
half, h2) phase at a time), software-pipelined: S(t+1) and interleaved
filler (V tiles, pair-1 q/k quarter-projections, c_proj tiles) are
emitted before the AV work that parks on exp(t). AV runs flipped
(stationary P^T 128-col slice, moving V' -> out [tq, 65]) so each AV
matmul costs 65 PE cycles instead of ~512, and the softmax denominator
lands as a per-partition column: normalization is reciprocal +
tensor_scalar on DVE, then a PE transpose (identity moving operand)
rebuilds y^T for c_proj.

Per-core layouts (partition dim first):
  xT      (1024, 2048)  x[b]^T bf16; SBUF as 8 chunks (128, 2048)
  q^T/k^T (256, 2048)   bf16, 2 chunks each; chunk p = heads 2p, 2p+1;
                        1/sqrt(hs) folded into Wq,bq
  V'      (2048, 4, 65) bf16, natural + ones column (fused softmax
                        denominator: AV matmul emits [O^T; l] with M=65)
  S^T     (tk, tq) PSUM fp32; exp on ScalarE (no max subtraction: logits
                        ~N(0,1), exp cannot overflow); P^T bf16 in SBUF
  diag    strict-lower triangle of the diagonal 128x128 block is zeroed
                        by a bf16 upper-tri mask multiply on the DVE
  y^T     (256, 2048)   bf16 normalized attention out
  out     (2048, 1024)  bf16 partial y@w_proj; host sums in fp32
"""

import numpy as np
import ml_dtypes

import concourse.tile as tile
from concourse import bacc, mybir
from concourse.bass_utils import run_bass_kernel_spmd

B, T, C = 2, 2048, 1024
HS = 64
NCORES = 8
NHL = 4            # heads per core
TCH = 512          # tq / projection T chunk
NT = T // 128      # 16 tk tiles
F32 = mybir.dt.float32
BF16 = mybir.dt.bfloat16


def build_program():
    nc = bacc.Bacc("TRN2", target_bir_lowering=False, debug=False)

    xT_d = nc.dram_tensor("xT", [C, T], BF16, kind="ExternalInput").ap()
    wqk_d = nc.dram_tensor("wqk", [C, 512], BF16, kind="ExternalInput").ap()
    wv_d = nc.dram_tensor("wv", [C, 256], BF16, kind="ExternalInput").ap()
    wp_d = nc.dram_tensor("wp", [256, 1024], BF16, kind="ExternalInput").ap()
    bqk_d = nc.dram_tensor("bqk", [128, 4], F32, kind="ExternalInput").ap()
    bvb_d = nc.dram_tensor("bvb", [128, 320], BF16, kind="ExternalInput").ap()
    msk_d = nc.dram_tensor("msk", [128, 128], BF16, kind="ExternalInput").ap()
    idn_d = nc.dram_tensor("idn", [128, 128], BF16, kind="ExternalInput").ap()
    out_d = nc.dram_tensor("out", [T, C], BF16, kind="ExternalOutput").ap()

    with tile.TileContext(nc) as tc:
        _kernel(tc, out_d, xT_d, wqk_d, wv_d, wp_d, bqk_d, bvb_d, msk_d, idn_d)
    nc.compile()
    return nc


def _kernel(tc, out_d, xT_d, wqk_d, wv_d, wp_d, bqk_d, bvb_d, msk_d, idn_d):
    nc = tc.nc
    AF = mybir.ActivationFunctionType

    with (
        tc.tile_pool(name="persist", bufs=1) as pers,
        tc.tile_pool(name="ps", bufs=2, space="PSUM") as ps,
        tc.tile_pool(name="po", bufs=4, space="PSUM") as po,
    ):
        # Weights on the Activation HWDGE queue (bqk + wqk lead so the
        # first q-projection matmul only waits for wqk[0] + xt[0]); x^T
        # streams on the sync HWDGE queue in parallel.
        xp = tc.alloc_tile_pool(name="xp", bufs=1)
        xT3 = xT_d.rearrange("(c p) t -> c p t", p=128)
        wqk3 = wqk_d.rearrange("(c p) m -> c p m", p=128)
        wv3 = wv_d.rearrange("(c p) m -> c p m", p=128)
        wp3 = wp_d.rearrange("(c p) m -> c p m", p=128)
        xt, wqk, wv = [], [], []
        for c in range(8):
            w_ = pers.tile([128, 512], BF16, tag=f"wqk{c}", name=f"wqk{c}")
            nc.scalar.dma_start(out=w_, in_=wqk3[c])
            wqk.append(w_)
            t_ = xp.tile([128, T], BF16, tag=f"xt{c}", name=f"xt{c}")
            nc.sync.dma_start(out=t_, in_=xT3[c])
            xt.append(t_)
        bqk = pers.tile([128, 4], F32, tag="bqk")
        nc.scalar.dma_start(out=bqk, in_=bqk_d)
        for c in range(8):
            t_ = pers.tile([128, 256], BF16, tag=f"wv{c}", name=f"wv{c}")
            nc.scalar.dma_start(out=t_, in_=wv3[c])
            wv.append(t_)
        bvb = pers.tile([128, 320], BF16, tag="bvb")
        nc.scalar.dma_start(out=bvb, in_=bvb_d)
        msk = pers.tile([128, 128], BF16, tag="msk")
        nc.scalar.dma_start(out=msk, in_=msk_d)
        idn = pers.tile([128, 128], BF16, tag="idn")
        nc.scalar.dma_start(out=idn, in_=idn_d)
        wp = []
        for c in range(2):
            t_ = pers.tile([128, 1024], BF16, tag=f"wp{c}", name=f"wp{c}")
            nc.scalar.dma_start(out=t_, in_=wp3[c])
            wp.append(t_)

        # zero operands for PE-warmup matmuls: accumulate +0 into live
        # PSUM tiles purely to keep the tensor engine's p-state ramp hot
        # across DMA-paced stretches (an idle PE drops back to half clock
        # for the next 3us).
        zs = pers.tile([64, 128], BF16, tag="zs")
        nc.gpsimd.memset(zs[:], 0)
        zs2 = pers.tile([64, 512], BF16, tag="zs2")
        nc.gpsimd.memset(zs2[:], 0)

        def warm(pst, n=1, cols=512, first_start=False):
            for i in range(n):
                nc.tensor.matmul(
                    pst[0:128, 0:cols], zs[:], zs2[:, 0:cols],
                    start=(first_start and i == 0), stop=False,
                    skip_group_check=True,
                )

        # q^T / k^T chunks: m=0,1 -> q pairs, m=2,3 -> k pairs
        qk = [pers.tile([128, T], BF16, tag=f"qk{m}", name=f"qk{m}")
              for m in range(4)]
        # V' = [V | 1] per (tk-tile, head); ones column from bvb[:, 256:320]
        v_all = pers.tile([128, NT, NHL, HS + 1], BF16, tag="v_all",
                          name="v_all")
        nc.vector.tensor_copy(
            out=v_all[:, :, :, HS],
            in_=bvb[:, 256:320].rearrange("p (a b) -> p a b", a=NT),
        )
        # y^T chunks (normalized attention output), pair-stacked
        yt = [pers.tile([128, T], BF16, tag=f"yt{p}", name=f"yt{p}")
              for p in range(2)]

        # ---- QKV projection: qk[m] = (x @ wqk[:, m-chunk])^T + bias.
        # mq (po pool) and mk (ps pool) are emitted interleaved per x
        # chunk so the PE keeps pace with the x DMA stream. ----
        def qk_proj_pair(mq, mk, warmups=False):
            pq = [po.tile([128, TCH], F32, tag="po", name=f"pst{mq}_{i}")
                  for i in range(4)]
            wide = [ps.tile([128, 1024], F32, tag="st", name=f"pw{mk}_{i}")
                    for i in range(2)]
            pk = [wide[i // 2][:, TCH * (i % 2):TCH * (i % 2 + 1)]
                  for i in range(4)]
            for c in range(8):
                if warmups:
                    warm(pq[0][:], n=(6 if c == 0 else 0),
                         first_start=(c == 0))
                for m, pst in ((mq, pq), (mk, pk)):
                    lhsT = wqk[c][:, 128 * m:128 * (m + 1)]
                    for i in range(4):
                        nc.tensor.matmul(
                            pst[i][:],
                            lhsT,
                            xt[c][:, TCH * i:TCH * (i + 1)],
                            start=(c == 0),
                            stop=(c == 7),
                        )
            for m, pst in ((mq, pq), (mk, pk)):
                for i in range(4):
                    if i % 2 == 0:
                        nc.vector.tensor_scalar_add(
                            out=qk[m][:, TCH * i:TCH * (i + 1)],
                            in0=pst[i][:],
                            scalar1=bqk[:, m:m + 1],
                        )
                    else:
                        nc.scalar.activation(
                            out=qk[m][:, TCH * i:TCH * (i + 1)],
                            in_=pst[i][:],
                            func=AF.Identity,
                            bias=bqk[:, m:m + 1],
                            scale=1.0,
                        )

        # ---- V tile t in natural layout (+bias) ----
        def v_tile(t):
            vp = po.tile([128, 256], F32, tag="po", name=f"vp{t}")
            for c in range(8):
                nc.tensor.matmul(
                    vp[:],
                    xt[c][:, 128 * t:128 * (t + 1)],
                    wv[c][:],
                    start=(c == 0),
                    stop=(c == 7),
                )
            nc.vector.tensor_add(
                out=v_all[:, t, :, 0:HS],
                in0=vp[:].rearrange("p (h d) -> p h d", h=NHL),
                in1=bvb[:, 0:256].rearrange("p (h d) -> p h d", h=NHL),
            )

        # ---- fill queue: self-contained ~0.4-0.9us PE work units pumped
        # one per attention step, so the in-order PE queue always has
        # ready work behind the exp-gated AV matmuls ----
        fill_q = []

        def pump(k=1):
            for _ in range(k):
                if fill_q:
                    fill_q.pop(0)()

        # one 256-col slice of a late q/k projection chunk m: full c
        # accumulation in a single po slot
        def qk_slice(m, i):
            pst = po.tile([128, 256], F32, tag="po", name=f"qs{m}_{i}")
            for c in range(8):
                nc.tensor.matmul(
                    pst[:],
                    wqk[c][:, 128 * m:128 * (m + 1)],
                    xt[c][:, 256 * i:256 * (i + 1)],
                    start=(c == 0),
                    stop=(c == 7),
                )
            nc.vector.tensor_scalar_add(
                out=qk[m][:, 256 * i:256 * (i + 1)],
                in0=pst[:],
                scalar1=bqk[:, m:m + 1],
            )

        # attention-side SBUF pools
        ptp = tc.alloc_tile_pool(name="pt", bufs=36)
        lrp = tc.alloc_tile_pool(name="lrec", bufs=6)
        ynp = tc.alloc_tile_pool(name="ynp", bufs=40)
        ostp = tc.alloc_tile_pool(name="ost", bufs=4)
        # y_norm2[pair][j]: [128 tq, 2 h2, 64 hs] bf16, filled by the two
        # h2 phases of a pair, then PE-transposed into yt[pair]
        yn = [[None] * NT for _ in range(2)]

        def drain_j(pair, h2, j, av):
            # l sits at column 64 of the flipped AV output: per-partition,
            # so normalization is a reciprocal + tensor_scalar multiply
            if h2 == 0:
                yn[pair][j] = ynp.tile([128, 2, HS], BF16, tag="yn",
                                       name=f"yn{pair}{j}")
            lr = lrp.tile([128, 1], F32, tag="lr", name=f"lr{pair}{h2}{j}")
            nc.vector.reciprocal(out=lr[:], in_=av[:, HS:HS + 1])
            nc.vector.tensor_scalar_mul(
                out=yn[pair][j][:, h2, :],
                in0=av[:, 0:HS],
                scalar1=lr[:],
            )
            if h2 == 1:
                pending_tp.append((pair, j))

        pending_tp = []

        def flush_tp():
            # transpose [128,128] into y^T via the PE (identity moving
            # operand), staged through PSUM; deferred a step so the PE
            # never parks waiting on the drain's DVE chain
            while pending_tp:
                pair, j = pending_tp.pop(0)
                tp = po.tile([128, 128], BF16, tag="po", name=f"tp{pair}{j}")
                nc.tensor.matmul(
                    tp[:],
                    yn[pair][j].rearrange("p a b -> p (a b)"),
                    idn[:],
                    start=True, stop=True, is_transpose=True,
                )
                nc.vector.tensor_copy(
                    out=yt[pair][:, 128 * j:128 * (j + 1)], in_=tp[:])

        # ---- c_proj partial tile t: out rows [128t, 128t+128) ----
        stg_live = {}

        def proj_half(t, oc, tail=False):
            if oc == 0:
                stg_live[t] = ostp.tile([128, 1024], BF16, tag="stg",
                                        name=f"stg{t}")
            stg = stg_live[t]
            pp = po.tile([128, TCH], F32, tag="po", name=f"pp{t}{oc}")
            for p2 in range(2):
                nc.tensor.matmul(
                    pp[:],
                    yt[p2][:, 128 * t:128 * (t + 1)],
                    wp[p2][:, TCH * oc:TCH * (oc + 1)],
                    start=(p2 == 0),
                    stop=(p2 == 1),
                )
            if tail and oc == 1:
                # Act is idle at the tail; split copies across engines
                nc.scalar.copy(out=stg[:, TCH:2 * TCH], in_=pp[:])
            else:
                nc.vector.tensor_copy(
                    out=stg[:, TCH * oc:TCH * (oc + 1)], in_=pp[:])
            if oc == 1:
                nc.sync.dma_start(out=out_d[128 * t:128 * (t + 1), :],
                                  in_=stg[:])
                del stg_live[t]

        def proj_tile(t, tail=False):
            proj_half(t, 0, tail)
            proj_half(t, 1, tail)

        # ---- attention phase for one (pair, half, h2): head-serial so
        # only 2 opr banks are live, leaving po slots for interleaved
        # proj/V work. extra(t) emits interleaved PE work after step t. ----
        def emit_S0(pair, half, h2):
            # first S tile of a phase, emitted inside the *previous* phase's
            # last step so the Act engine never starves across boundaries
            pb = 64 * h2
            st = ps.tile([128, 1024], F32, tag="st",
                         name=f"s0{pair}{half}{h2}")
            for cc in range(2):
                nc.tensor.matmul(
                    st[:, TCH * cc:TCH * (cc + 1)],
                    qk[2 + pair][pb:pb + 64, 0:128],
                    qk[pair][pb:pb + 64,
                             TCH * (2 * half + cc):TCH * (2 * half + cc + 1)],
                    start=True,
                    stop=True,
                )
            return st

        def attn(pair, half, h2, extra=None, st0=None, prelude=None):
            t_end = 8 * (half + 1)
            h = 2 * pair + h2
            pb = 64 * h2
            pts = {}

            def emit_S(t):
                # row-packed K=64 matmul: head h2 lives at partitions
                # 64*h2..64*h2+64 of the qk chunks; moving dim trimmed
                # to the causal boundary (no small-N penalty in bf16).
                st = ps.tile([128, 1024], F32, tag="st",
                             name=f"st{pair}{half}{t}{h2}")
                for cc in range(2):
                    cg = 2 * half + cc
                    if cg < t // 4:
                        continue
                    sub0 = max(0, 128 * t - TCH * cg)
                    nc.tensor.matmul(
                        st[:, TCH * cc + sub0:TCH * (cc + 1)],
                        qk[2 + pair][pb:pb + 64, 128 * t:128 * (t + 1)],
                        qk[pair][pb:pb + 64, TCH * cg + sub0:TCH * (cg + 1)],
                        start=True,
                        stop=True,
                    )
                return st

            # software-pipelined emission: the PE queue is in-order, so
            # S(t+1) and any interleaved filler must be emitted BEFORE
            # AV(t), which parks waiting on exp(t).
            st = st0 if st0 is not None else emit_S(0)
            pre = None
            for t in range(t_end):
                rel = max(128 * t, 1024 * half) - 1024 * half
                pt = ptp.tile([128, 1024], BF16, tag="pt",
                              name=f"pt{pair}{half}{t}{h2}")
                nc.scalar.activation(
                    out=pt[:, rel:1024], in_=st[:, rel:1024], func=AF.Exp
                )
                if t + 1 < t_end:
                    st = emit_S(t + 1)
                elif prelude is not None:
                    pre = prelude()
                pump(1)
                flush_tp()
                if t // 8 == half:
                    # zero strict-lower triangle (tk > tq) of diag block
                    nc.vector.tensor_mul(
                        out=pt[:, rel:rel + 128],
                        in0=pt[:, rel:rel + 128],
                        in1=msk[:],
                    )
                pts[t] = pt
                # flipped AV, j-major burst: once exp(t) lands, the output
                # tile for tq-tile j == t is fully determined; accumulate
                # it over all pt(t' <= t) in one go. Stationary P^T 128-col
                # slice, moving V' [128, 65] -> out [tq, 65]: 65-cycle
                # matmuls, and l lands as a per-partition column.
                jj = t - 8 * half
                if jj >= 0:
                    avt = po.tile([128, HS + 1], F32, tag="po",
                                  name=f"av{pair}{half}{h2}{jj}")
                    for tp_ in range(t + 1):
                        nc.tensor.matmul(
                            avt[:],
                            pts[tp_][:, 128 * jj:128 * (jj + 1)],
                            v_all[:, tp_, h, :],
                            start=(tp_ == 0),
                            stop=(tp_ == t),
                        )
                    drain_j(pair, h2, t, avt)
                if extra is not None:
                    extra(t)
            flush_tp()
            return pre

        def attn2(pair, half, extra=None, st0=None):
            t_end = 8 * (half + 1)
            h0 = 2 * pair
            pts = {}

            def emit_S2(t, h2):
                st = ps.tile([128, 1024], F32, tag="st",
                             name=f"s2{pair}{half}{t}{h2}")
                pb = 64 * h2
                for cc in range(2):
                    cg = 2 * half + cc
                    if cg < t // 4:
                        continue
                    sub0 = max(0, 128 * t - TCH * cg)
                    nc.tensor.matmul(
                        st[:, TCH * cc + sub0:TCH * (cc + 1)],
                        qk[2 + pair][pb:pb + 64, 128 * t:128 * (t + 1)],
                        qk[pair][pb:pb + 64, TCH * cg + sub0:TCH * (cg + 1)],
                        start=True,
                        stop=True,
                    )
                return st

            st2 = [st0 if st0 is not None else emit_S2(0, 0),
                   emit_S2(0, 1)]
            for t in range(t_end):
                rel = max(128 * t, 1024 * half) - 1024 * half
                for h2 in range(2):
                    pt = ptp.tile([128, 1024], BF16, tag="pt",
                                  name=f"p2{pair}{half}{t}{h2}")
                    nc.scalar.activation(
                        out=pt[:, rel:1024], in_=st2[h2][:, rel:1024],
                        func=AF.Exp,
                    )
                    if t + 1 < t_end:
                        st2[h2] = emit_S2(t + 1, h2)
                    pump(1)
                    if t // 8 == half:
                        nc.vector.tensor_mul(
                            out=pt[:, rel:rel + 128],
                            in0=pt[:, rel:rel + 128],
                            in1=msk[:],
                        )
                    pts[(t, h2)] = pt
                    jj = t - 8 * half
                    if jj >= 0:
                        avt = po.tile([128, HS + 1], F32, tag="po",
                                      name=f"a2{pair}{half}{h2}{jj}")
                        for tp_ in range(t + 1):
                            nc.tensor.matmul(
                                avt[:],
                                pts[(tp_, h2)][:, 128 * jj:128 * (jj + 1)],
                                v_all[:, tp_, h0 + h2, :],
                                start=(tp_ == 0),
                                stop=(tp_ == t),
                            )
                        drain_j(pair, h2, t, avt)
                    flush_tp()
                if extra is not None:
                    extra(t)

        # -------- schedule --------
        # Every attention phase is exp(Act)-bound; all remaining PE work
        # (V tiles, pair-1 q/k quarters, c_proj tiles) is interleaved into
        # those phases the moment its dependencies allow.
        def mk_extra(fns, at):
            sched = dict(zip(at, fns))
            return lambda t: sched[t]() if t in sched else None

        qk_proj_pair(0, 2, warmups=True)  # pair-0 q/k, paced by the x stream
        for t in range(3):
            v_tile(t)

        def mk_extra(fns, at):
            sched = dict(zip(at, fns))
            return lambda t: sched[t]() if t in sched else None

        # pair-0 half-0 attention; V tiles 3..15 stream through its exp
        # gaps three steps ahead of their first AV use
        s0 = attn(0, 0, 0,
                  extra=lambda t: v_tile(3 + t) if t < 8 else None,
                  prelude=lambda: emit_S0(0, 0, 1))
        s0 = attn(0, 0, 1,
                  extra=lambda t: v_tile(11 + t) if t < 5 else None,
                  st0=s0, prelude=lambda: emit_S0(0, 1, 0))

        # pair-0 half-1 attention; pair-1 q/k projection slices ride in
        # its exp gaps
        s0 = attn(0, 1, 0, extra=mk_extra(
            [lambda i=i: qk_slice(1, i) for i in range(4)] +
            [lambda i=i: qk_slice(3, i) for i in range(4)],
            [1, 3, 5, 7, 9, 11, 13, 15]),
            st0=s0, prelude=lambda: emit_S0(0, 1, 1))
        s0 = attn(0, 1, 1, extra=mk_extra(
            [lambda i=i: qk_slice(1, i) for i in range(4, 8)] +
            [lambda i=i: qk_slice(3, i) for i in range(4, 8)],
            [1, 3, 5, 7, 9, 11, 13, 15]),
            st0=s0, prelude=lambda: emit_S0(1, 0, 0))

        s0 = attn(1, 0, 0, st0=s0, prelude=lambda: emit_S0(1, 0, 1))
        # c_proj half-tiles interleave as soon as their yt columns are
        # complete (tiles 0-3 need only the first 512 tq columns)
        s0 = attn(1, 0, 1, extra=mk_extra(
            [lambda t=t, oc=oc: proj_half(t, oc)
             for t in (0, 1) for oc in (0, 1)], [4, 5, 6, 7]),
            st0=s0, prelude=lambda: emit_S0(1, 1, 0))
        p2sched = {k: [lambda t=2 + k, oc=oc: proj_half(t, oc)
                       for oc in (0, 1)] for k in range(6)}
        for k in range(6):
            # late merged-phase steps: exp is short there, so the oc=1
            # staging copy goes to the otherwise-idle Act engine
            p2sched[10 + k] = [lambda t=8 + k, oc=oc: proj_half(t, oc, True)
                               for oc in (0, 1)]
        attn2(1, 1, extra=lambda t: [f() for f in p2sched.get(t, [])], st0=s0)
        proj_tile(14, tail=True)
        proj_tile(15, tail=True)
        ostp.release()
        ynp.release()
        lrp.release()
        ptp.release()
        xp.release()


_PROG = None


def _get_program():
    global _PROG
    if _PROG is None:
        _PROG = build_program()
    return _PROG


def _bf(a):
    return np.ascontiguousarray(np.asarray(a, dtype=ml_dtypes.bfloat16))


def make_in_maps(x, w_attn, b_attn, w_proj, b_proj):
    x = np.asarray(x, dtype=np.float32)
    w_attn = np.asarray(w_attn, dtype=np.float32)
    b_attn = np.asarray(b_attn, dtype=np.float32)
    w_proj = np.asarray(w_proj, dtype=np.float32)
    s = 1.0 / np.sqrt(HS)
    wq, wk, wv = w_attn[:, 0:C], w_attn[:, C:2 * C], w_attn[:, 2 * C:3 * C]
    bq, bk, bv = b_attn[0:C], b_attn[C:2 * C], b_attn[2 * C:3 * C]
    # upper-triangular-inclusive causal mask for the S^T diagonal block
    msk = np.triu(np.ones((128, 128), dtype=np.float32))
    in_maps = []
    for core in range(NCORES):
        b, g = divmod(core, 4)
        cs = slice(256 * g, 256 * (g + 1))
        bqk_ = np.concatenate([bq[cs] * s, bk[cs]]).reshape(4, 128).T.copy()
        in_maps.append({
            "xT": _bf(x[b].T),
            "wqk": _bf(np.concatenate([wq[:, cs] * s, wk[:, cs]], axis=1)),
            "wv": _bf(wv[:, cs]),
            "wp": _bf(w_proj[cs, :]),
            "bqk": np.ascontiguousarray(bqk_),
            "bvb": _bf(np.concatenate([
                np.broadcast_to(bv[cs][None, :], (128, 256)),
                np.ones((128, 64), dtype=np.float32)], axis=1)),
            "msk": _bf(msk),
            "idn": _bf(np.eye(128, dtype=np.float32)),
        })
    return in_maps


def gather_output(results, b_proj):
    b_proj = np.asarray(b_proj, dtype=np.float32)
    out = np.empty((B, T, C), dtype=np.float32)
    for b in range(B):
        acc = results[4 * b]["out"].astype(np.float32)
        for g in range(1, 4):
            acc = acc + results[4 * b + g]["out"].astype(np.float32)
        out[b] = acc + b_proj[None, :]
    return out


def kernel(x, w_attn, b_attn, w_proj, b_proj):
    nc = _get_program()
    in_maps = make_in_maps(x, w_attn, b_attn, w_proj, b_proj)
    res = run_bass_kernel_spmd(nc, in_maps, core_ids=list(range(NCORES)))
    return gather_output(res.results, b_proj)
